# revision 14
# baseline (speedup 1.0000x reference)
"""Trainium2 Bass kernel for nn_Decoder (2-block transformer decoder layer).

Sharding: 8 cores; core c owns batch b=c//4 and query-token slice qi=c%4
(512 of 2048 tokens). Each core loads its batch's full x/context (needed for
K/V) plus all weights; no collectives. Output slice [512, 512] per core.

Within-core dataflow (all matmuls bf16, fp32 PSUM accumulation):
  - activations kept feature-major (x^T: [D-part, token-free]) for
    projections; scores computed transposed (S^T: [key-part, query-free])
    so softmax-exp output feeds the AV matmul with zero transposes.
  - K=64 score matmuls packed 2 heads/array via PE row tiling;
    AV (M=64) packed 2 heads via column tiling; sum-of-exp via ones-vector
    matmuls col-tiled at partition offsets {0,32}.
  - softmax normalization deferred to after AV (scale rows by 1/sumexp).
  - per-channel bias adds folded into PSUM as rank-1 (ones x bias-row)
    matmuls; per-head q/k biases as per-partition tensor_scalar adds.
  - LayerNorm token-major in fp32: bn_stats/bn_aggr, rstd = Exp(-0.5*Ln(v+eps))
    (same ACT table set as softmax's Exp => no table switches).
"""

import numpy as np

import concourse.bass as bass
import concourse.mybir as mybir
import concourse.tile as tile
from concourse import bacc
from concourse.bass_utils import run_bass_kernel_spmd

F32 = mybir.dt.float32
BF16 = mybir.dt.bfloat16
AF = mybir.ActivationFunctionType
OP = mybir.AluOpType

P = 128
B, S, D = 2, 2048, 512
H, R = 8, 64
F = 4 * D
SQ = 512          # queries per core
DO = D // P       # 4  d-subtiles
KO = (H * R) // P  # 4  hr-subtiles
STO = S // P      # 16 key-token subtiles
SQO = SQ // P     # 4  query-token subtiles
FO = F // P       # 16 ffn-hidden subtiles
PAIRS = H // 2    # 4
EPS = 1e-3
SCALE = 1.0 / np.sqrt(R).astype(np.float32)  # 0.125, folded into exp


def _emit(nc, tc, T, ctxstack, dbg=None):
    ten = nc.tensor
    ec = ctxstack.enter_context

    def tap(name, ap):
        # pop => only the first producer writes a given debug tensor
        if dbg is not None and name in dbg:
            nc.sync.dma_start(dbg.pop(name)[:], ap)

    # ---------------- pools ----------------
    wp = ec(tc.tile_pool(name="wp", bufs=1))
    mha = ec(tc.tile_pool(name="mha", bufs=1))
    exps = ec(tc.tile_pool(name="exps", bufs=3))
    resid = ec(tc.tile_pool(name="resid", bufs=2))
    xts = ec(tc.tile_pool(name="xts", bufs=2))
    bf4 = ec(tc.tile_pool(name="bf4", bufs=1))
    hbp = ec(tc.tile_pool(name="hbp", bufs=3))
    ln = ec(tc.tile_pool(name="ln", bufs=3))
    recp = ec(tc.tile_pool(name="recp", bufs=1))
    drs = ec(tc.tile_pool(name="drs", bufs=2, space="DRAM"))
    ps_mm = ec(tc.tile_pool(name="ps_mm", bufs=2, space="PSUM"))
    ps_sc = ec(tc.tile_pool(name="ps_sc", bufs=2, space="PSUM"))
    ps_acc = ec(tc.tile_pool(name="ps_acc", bufs=1, space="PSUM"))

    ones_bf = wp.tile([P, 64], BF16, tag="ones")
    nc.vector.memset(ones_bf[:], 1.0)
    ones_row = wp.tile([1, P], BF16, tag="ones_row")
    nc.vector.memset(ones_row[:], 1.0)
    eps_col = wp.tile([P, 1], F32, tag="eps")
    nc.vector.memset(eps_col[:], EPS)

    def bias_mm(ps_ap, brow_ap):
        """psum += ones^T(1xM) @ bias_row(1xN): broadcast-add a row vector."""
        m = ps_ap.shape[0]
        ten.matmul(ps_ap, ones_row[:, :m], brow_ap, start=False, stop=True)

    # ---------------- weights: load fp32, cast to bf16 ----------------
    with tc.tile_pool(name="stg", bufs=2) as stg:
        def qkv_w(name):
            # [H, D, R] -> sbuf [P, DO, H, R]  (partition = d%128)
            st = stg.tile([P, DO, H, R], F32, tag="wstg")
            for do in range(DO):
                nc.sync.dma_start(
                    st[:, do],
                    T[name][:, do * P:(do + 1) * P, :].rearrange("h p r -> p h r"),
                )
            w = wp.tile([P, DO, H, R], BF16, tag=f"{name}_bf")
            nc.any.tensor_copy(out=w[:], in_=st[:])
            return w

        def wo_w(name):
            # [H*R, D] -> [P, KO, D]
            st = stg.tile([P, KO, D], F32, tag="wstg")
            nc.sync.dma_start(st[:], T[name][:].rearrange("(ko p) d -> p ko d", p=P))
            w = wp.tile([P, KO, D], BF16, tag=f"{name}_bf")
            nc.any.tensor_copy(out=w[:], in_=st[:])
            return w

        wq1, wk1, wv1 = qkv_w("Wq1"), qkv_w("Wk1"), qkv_w("Wv1")
        wq2, wk2, wv2 = qkv_w("Wq2"), qkv_w("Wk2"), qkv_w("Wv2")
        wo1, wo2 = wo_w("Wo1"), wo_w("Wo2")

        # W1 [D, F] -> [P, DO, FO, P] ; chunked by do to bound staging
        w1 = wp.tile([P, DO, FO, P], BF16, tag="W1_bf")
        for do in range(DO):
            st = stg.tile([P, FO, P], F32, tag="wstg")
            nc.sync.dma_start(
                st[:],
                T["W1"][do * P:(do + 1) * P, :].rearrange("p (fo fi) -> p fo fi", fo=FO),
            )
            nc.any.tensor_copy(out=w1[:, do], in_=st[:])

        # W2 [F, D] -> [P, FO, D] ; chunked by fo-quarters
        w2 = wp.tile([P, FO, D], BF16, tag="W2_bf")
        for c in range(4):
            st = stg.tile([P, 4, D], F32, tag="wstg")
            nc.sync.dma_start(
                st[:],
                T["W2"][:].rearrange("(fo p) d -> p fo d", p=P)[:, c * 4:(c + 1) * 4],
            )
            nc.any.tensor_copy(out=w2[:, c * 4:(c + 1) * 4], in_=st[:])

        # per-partition bias columns: bq/bk [H, R] -> [(half r)=128, pair=4]
        def qk_bias(name):
            t = wp.tile([P, PAIRS], F32, tag=f"{name}_sb")
            nc.sync.dma_start(
                t[:], T[name][:].rearrange("(pair half) r -> (half r) pair", half=2)
            )
            return t

        bq1s, bk1s = qk_bias("bq1"), qk_bias("bk1")
        bq2s, bk2s = qk_bias("bq2"), qk_bias("bk2")

        # row-vector biases, each its own [1, 512] bf16 tile at base partition 0
        def bias_row(name, flatten=False):
            src = T[name][:]
            if flatten:
                src = src.rearrange("h r -> (h r)")
            st = stg.tile([1, D], F32, tag="wstg")
            nc.sync.dma_start(st[:], src[None, :])
            t = wp.tile([1, D], BF16, tag=f"{name}_row")
            nc.any.tensor_copy(out=t[:], in_=st[:])
            return t[:]

        bv1r = bias_row("bv1", flatten=True)
        bv2r = bias_row("bv2", flatten=True)
        bo1r = bias_row("bo1")
        bo2r = bias_row("bo2")
        b2r = bias_row("b2")

        # LN gamma/beta as full [P, 512] fp32 (elementwise along free dim)
        gs = wp.tile([P, D], F32, tag="g_sb")
        nc.sync.dma_start(gs[:], T["ln_g"][:][None, :].to_broadcast([P, D]))
        bs = wp.tile([P, D], F32, tag="b_sb")
        nc.sync.dma_start(bs[:], T["ln_b"][:][None, :].to_broadcast([P, D]))

        b1s = wp.tile([P, FO], F32, tag="b1_sb")
        nc.sync.dma_start(b1s[:], T["b1"][:].rearrange("(fo p) -> p fo", p=P))

        # ---------------- activations: fp32 -> bf16 -> DRAM scratch ----------
        def to_bf16_dram(name, n_rows):
            xdr = drs.tile([n_rows, D], BF16, tag="xdr")
            for c in range(n_rows // SQ):
                st = stg.tile([P, SQO, D], F32, tag="wstg")
                nc.sync.dma_start(
                    st[:],
                    T[name][c * SQ:(c + 1) * SQ, :].rearrange("(so p) d -> p so d", p=P),
                )
                cb = bf4.tile([P, SQO, D], BF16, tag="cast4")
                nc.any.tensor_copy(out=cb[:], in_=st[:])
                nc.sync.dma_start(
                    xdr[c * SQ:(c + 1) * SQ, :].rearrange("(so p) d -> p so d", p=P),
                    cb[:],
                )
            return xdr

        x_dr = to_bf16_dram("xb", S)
        ctx_dr = to_bf16_dram("ctx", S)

        # query slice: keep fp32 natural (residual source)
        xq_nat = wp.tile([P, SQO, D], F32, tag="xq_nat")
        nc.sync.dma_start(xq_nat[:], T["xq"][:].rearrange("(so p) d -> p so d", p=P))

    def load_kvT(xdr):
        # DRAM [S, D] bf16 -> feature-major [P, DO, S] via xbar transpose
        kvT = wp.tile([P, DO, S], BF16, tag="kvT")
        for do in range(DO):
            nc.sync.dma_start_transpose(kvT[:, do], xdr[:, do * P:(do + 1) * P])
        return kvT

    def transpose_small(src_bf16_4):
        """[P, SQO, D] bf16 token-major -> [P, DO, SQ] bf16 feature-major."""
        xT = xts.tile([P, DO, SQ], BF16, tag="xT_small")
        sdr = drs.tile([SQ, D], BF16, tag="sdr")
        nc.sync.dma_start(sdr[:].rearrange("(so p) d -> p so d", p=P), src_bf16_4[:])
        for do in range(DO):
            nc.sync.dma_start_transpose(xT[:, do], sdr[:, do * P:(do + 1) * P])
        return xT

    xq_bf = bf4.tile([P, SQO, D], BF16, tag="cast4")
    nc.any.tensor_copy(out=xq_bf[:], in_=xq_nat[:])
    xq_T = transpose_small(xq_bf)

    # ---------------- one MHA block ----------------
    def mha_block(kv_T, q_T, wq, bqs, wk, bks, wv, bvr, wo, bor, resid_nat):
        # K^T: [P(2-head R), pair, S]
        kT = mha.tile([P, PAIRS, S], BF16, tag="kT")
        for pair in range(PAIRS):
            for st4 in range(STO // 4):
                ps = ps_mm.tile([P, 512], F32, tag="mm")
                for do in range(DO):
                    ten.matmul(
                        ps[:], wq_sl(wk, do, pair), kv_T[:, do, st4 * 512:(st4 + 1) * 512],
                        start=(do == 0), stop=(do == DO - 1),
                    )
                nc.any.tensor_scalar(
                    out=kT[:, pair, st4 * 512:(st4 + 1) * 512], in0=ps[:],
                    scalar1=bks[:, pair:pair + 1], scalar2=None, op0=OP.add,
                )

        # V token-major: [P(key token), so, H*R]
        v_sb = mha.tile([P, STO, H * R], BF16, tag="v")
        for so in range(STO):
            ps = ps_mm.tile([P, 512], F32, tag="mm")
            for do in range(DO):
                ten.matmul(
                    ps[:], kv_T[:, do, so * P:(so + 1) * P], wv[:, do],
                    start=(do == 0), stop=False,
                )
            bias_mm(ps[:], bvr)
            nc.any.tensor_copy(out=v_sb[:, so], in_=ps[:])

        # Q^T: [P(2-head R), pair, SQ]
        qT = mha.tile([P, PAIRS, SQ], BF16, tag="qT")
        for pair in range(PAIRS):
            ps = ps_mm.tile([P, 512], F32, tag="mm")
            for do in range(DO):
                ten.matmul(
                    ps[:], wq_sl(wq, do, pair), q_T[:, do],
                    start=(do == 0), stop=(do == DO - 1),
                )
            nc.any.tensor_scalar(
                out=qT[:, pair], in0=ps[:],
                scalar1=bqs[:, pair:pair + 1], scalar2=None, op0=OP.add,
            )

        tap("kT", kT[:])
        tap("v", v_sb[:])
        tap("qT", qT[:])
        # attention, one head-pair at a time
        oT = mha.tile([P, KO, SQ], BF16, tag="oT")
        for pair in range(PAIRS):
            av_ps = ps_acc.tile([P, 512], F32, tag="av")
            sum_ps = ps_acc.tile([P, 512], F32, tag="sums")
            for sgrp in range(STO // 2):
                for half in range(2):
                    hb = half * 64
                    sc = ps_sc.tile([P, 1024], F32, tag="scores")
                    for k2 in range(2):
                        st = sgrp * 2 + k2
                        ten.matmul(
                            sc[:, k2 * 512:(k2 + 1) * 512],
                            kT[hb:hb + 64, pair, st * P:(st + 1) * P],
                            qT[hb:hb + 64, pair],
                            start=True, stop=True,
                        )
                    et = exps.tile([P, 1024], BF16, tag="expT")
                    nc.scalar.activation(et[:], sc[:], AF.Exp, scale=float(SCALE))
                    if pair == 0 and sgrp == 0 and half == 0:
                        tap("exp00", et[:])
                    h = 2 * pair + half
                    for k2 in range(2):
                        st = sgrp * 2 + k2
                        ten.matmul(
                            av_ps[hb:hb + 64, :], v_sb[:, st, h * R:(h + 1) * R],
                            et[:, k2 * 512:(k2 + 1) * 512],
                            start=(st == 0), stop=(st == STO - 1),
                            skip_group_check=True,
                        )
                        ten.matmul(
                            sum_ps[hb:hb + 64, :], ones_bf[:],
                            et[:, k2 * 512:(k2 + 1) * 512],
                            start=(st == 0), stop=(st == STO - 1),
                            skip_group_check=True,
                        )
            # 1/sumexp -> broadcast to 64 rows each
            if pair == 0:
                sumc = recp.tile([P, 512], F32, tag="sumdump")
                nc.vector.tensor_copy(out=sumc[:], in_=sum_ps[:])
                tap("sum0", sumc[:])
                avc = recp.tile([P, 512], F32, tag="avdump")
                nc.vector.tensor_copy(out=avc[:], in_=av_ps[:])
                tap("av0", avc[:])
            recb = recp.tile([P, 512], F32, tag="recb")
            nc.vector.reciprocal_approx_fast(out=recb[:], in_=sum_ps[:])
            if pair == 0:
                tap("recb0", recb[:])
            for half in range(2):
                hb = half * 64
                nc.vector.tensor_tensor(
                    out=oT[hb:hb + 64, pair], in0=av_ps[hb:hb + 64, :],
                    in1=recb[hb:hb + 64, :], op=OP.mult,
                )

        # Wo projection + bias + residual -> pre (fp32 token-major)
        pre = resid.tile([P, SQO, D], F32, tag="xout")
        for sq in range(SQO):
            ps = ps_mm.tile([P, 512], F32, tag="mm")
            for ko in range(KO):
                ten.matmul(
                    ps[:], oT[:, ko, sq * P:(sq + 1) * P], wo[:, ko],
                    start=(ko == 0), stop=False,
                )
            bias_mm(ps[:], bor)
            nc.vector.tensor_tensor(
                out=pre[:, sq], in0=ps[:], in1=resid_nat[:, sq], op=OP.add
            )
        tap("oT", oT[:])
        tap("pre", pre[:])
        return pre

    def wq_sl(w, do, pair):
        return w[:, do, 2 * pair:2 * pair + 2]

    def layernorm(pre, want_T):
        xn = resid.tile([P, SQO, D], F32, tag="xout")
        for sq in range(SQO):
            st6 = ln.tile([P, 6], F32, tag="st6")
            nc.vector.bn_stats(st6[:], pre[:, sq])
            mv = ln.tile([P, 2], F32, tag="mv")
            nc.vector.bn_aggr(mv[:], st6[:])
            lnv = ln.tile([P, 1], F32, tag="lnv")
            nc.scalar.activation(lnv[:], mv[:, 1:2], AF.Ln, bias=eps_col[:])
            rstd = ln.tile([P, 1], F32, tag="rstd")
            nc.scalar.activation(rstd[:], lnv[:], AF.Exp, scale=-0.5)
            nc.vector.tensor_scalar(
                out=xn[:, sq], in0=pre[:, sq],
                scalar1=mv[:, 0:1], scalar2=rstd[:], op0=OP.subtract, op1=OP.mult,
            )
            nc.vector.tensor_tensor(out=xn[:, sq], in0=xn[:, sq], in1=gs[:], op=OP.mult)
            nc.vector.tensor_tensor(out=xn[:, sq], in0=xn[:, sq], in1=bs[:], op=OP.add)
        xn_T = None
        if want_T:
            xb = bf4.tile([P, SQO, D], BF16, tag="cast4")
            nc.any.tensor_copy(out=xb[:], in_=xn[:])
            xn_T = transpose_small(xb)
        return xn, xn_T

    # ---------------- the network ----------------
    x_T = load_kvT(x_dr)
    tap("xT", x_T[:])
    tap("xqT", xq_T[:])
    pre1 = mha_block(x_T, xq_T, wq1, bq1s, wk1, bk1s, wv1, bv1r, wo1, bo1r, xq_nat)
    x1, x1_T = layernorm(pre1, want_T=True)
    tap("x1", x1[:])

    ctx_T = load_kvT(ctx_dr)
    pre2 = mha_block(ctx_T, x1_T, wq2, bq2s, wk2, bk2s, wv2, bv2r, wo2, bo2r, x1)
    x2, x2_T = layernorm(pre2, want_T=True)

    # ---------------- FFN ----------------
    # y accumulators: 2 x [P,1024] psum tiles = 4 x [P,512] regions (one per sq)
    yA = ps_sc.tile([P, 1024], F32, tag="scores")
    yB = ps_sc.tile([P, 1024], F32, tag="scores")
    y_ps = [yA[:, 0:512], yA[:, 512:1024], yB[:, 0:512], yB[:, 512:1024]]
    for fo in range(FO):
        ps = ps_mm.tile([P, 512], F32, tag="mm")
        for do in range(DO):
            ten.matmul(
                ps[:], w1[:, do, fo], x2_T[:, do],
                start=(do == 0), stop=(do == DO - 1),
            )
        hb = hbp.tile([P, 512], BF16, tag="hchunk")
        nc.scalar.activation(hb[:], ps[:], AF.Relu, bias=b1s[:, fo:fo + 1])
        for sq in range(SQO):
            ten.matmul(
                y_ps[sq], hb[:, sq * P:(sq + 1) * P], w2[:, fo],
                start=(fo == 0), stop=False,
            )
    pre3 = resid.tile([P, SQO, D], F32, tag="xout")
    for sq in range(SQO):
        bias_mm(y_ps[sq], b2r)
        nc.vector.tensor_tensor(out=pre3[:, sq], in0=y_ps[sq], in1=x2[:, sq], op=OP.add)

    x3, _ = layernorm(pre3, want_T=False)
    nc.sync.dma_start(T["out"][:].rearrange("(so p) d -> p so d", p=P), x3[:])


_INPUT_SPECS = [
    ("xb", [S, D], F32), ("xq", [SQ, D], F32), ("ctx", [S, D], F32),
    ("Wq1", [H, D, R], F32), ("bq1", [H, R], F32),
    ("Wk1", [H, D, R], F32), ("bk1", [H, R], F32),
    ("Wv1", [H, D, R], F32), ("bv1", [H, R], F32),
    ("Wo1", [H * R, D], F32), ("bo1", [D], F32),
    ("Wq2", [H, D, R], F32), ("bq2", [H, R], F32),
    ("Wk2", [H, D, R], F32), ("bk2", [H, R], F32),
    ("Wv2", [H, D, R], F32), ("bv2", [H, R], F32),
    ("Wo2", [H * R, D], F32), ("bo2", [D], F32),
    ("ln_g", [D], F32), ("ln_b", [D], F32),
    ("W1", [D, F], F32), ("b1", [F], F32),
    ("W2", [F, D], F32), ("b2", [D], F32),
]


DBG_SPECS = {
    "xT": ([P, DO, S], BF16), "xqT": ([P, DO, SQ], BF16),
    "kT": ([P, PAIRS, S], BF16), "v": ([P, STO, H * R], BF16),
    "qT": ([P, PAIRS, SQ], BF16), "exp00": ([P, 1024], BF16),
    "sum0": ([P, 512], F32), "oT": ([P, KO, SQ], BF16),
    "pre": ([P, SQO, D], F32), "x1": ([P, SQO, D], F32),
    "av0": ([P, 512], F32), "recb0": ([P, 512], F32),
}


def build(n_iters=1, debug=()):
    from contextlib import ExitStack

    nc = bacc.Bacc(None)
    T = {}
    for name, shape, dt in _INPUT_SPECS:
        T[name] = nc.dram_tensor(name, shape, dt, kind="ExternalInput")
    T["out"] = nc.dram_tensor("out", [SQ, D], F32, kind="ExternalOutput")
    dbg = {}
    for name in debug:
        shape, dt = DBG_SPECS[name]
        dbg[name] = nc.dram_tensor("dbg_" + name, shape, dt, kind="ExternalOutput")

    with tile.TileContext(nc) as tc:
        with ExitStack() as ctxstack:
            if n_iters == 1:
                _emit(nc, tc, T, ctxstack, dbg=dbg)
            else:
                with tc.For_i(0, n_iters, 1):
                    _emit(nc, tc, T, ctxstack, dbg=dbg)
    nc.finalize()
    return nc


_NC_CACHE = {}


def _per_core_inputs(inputs, core):
    b, qi = divmod(core, 4)
    m = {
        "xb": np.ascontiguousarray(inputs["x"][b]),
        "xq": np.ascontiguousarray(inputs["x"][b, qi * SQ:(qi + 1) * SQ]),
        "ctx": np.ascontiguousarray(inputs["context"][b]),
    }
    for name, _, _ in _INPUT_SPECS[3:]:
        m[name] = np.ascontiguousarray(inputs[name])
    return m


def kernel(**inputs):
    inputs = {k: np.asarray(v, dtype=np.float32) for k, v in inputs.items()}
    if "nc" not in _NC_CACHE:
        _NC_CACHE["nc"] = build(1)
    nc = _NC_CACHE["nc"]
    in_maps = [_per_core_inputs(inputs, c) for c in range(8)]
    res = run_bass_kernel_spmd(nc, in_maps, list(range(8)))
    out = np.empty((B, S, D), np.float32)
    for c in range(8):
        b, qi = divmod(c, 4)
        out[b, qi * SQ:(qi + 1) * SQ] = res.results[c]["out"]
    return out
